# revision 3
# baseline (speedup 1.0000x reference)
"""Trainium2 Bass kernel for nn_AxisSimplestSpline — relu-basis rewrite.

Math (per batch b, axis a):  g = (f - mins)/dx in [0,17),  f = A^T raw.
  est_a(g) = Y0 + lin_a*g + sum_{k=1..16} d_k * basis_k(g)
with d_k the slope-diffs of the PWL and a *two-sided* relu basis to keep
fp16 feature magnitudes <= 8:
  k in 1..8 : basis = min(g-k, 0)   (weight -d_k)   [DVE/GPSIMD]
              or relu(k-g)          (weight +d_k)   [ACT]
  k in 9..16: basis = relu(g-k)     (weight +d_k)
The linear remainder d_k*(g-k) of the min-side knots folds into lin_a.

out[c] = sum_a pinv[a,c] est_a: the per-knot weights fuse d_k*pinv, the
linear term lin_a*g becomes one extra matmul on raw itself, and the
constant goes into the drain bias.  Engine split per block: ACT builds
g (fp32, SBUF) + 2 knot-pairs, DVE 4 pairs + PSUM drain, GPSIMD 2 pairs.
Each pair of knots shares one matmul (features stacked 2x(8a x 8j) =
K=128, M=24); 4 pixel blocks land on distinct PE column-groups via
tile_position so their matmuls overlap in the array.
"""

import sys

sys.path.insert(0, "/opt/trn_rl_repo")

import numpy as np

import concourse.bacc as bacc
import concourse.mybir as mybir
import concourse.tile as tile
from concourse.bass_utils import run_bass_kernel_spmd

F32 = mybir.dt.float32
F16 = mybir.dt.float16
EPS = 1e-4
B, C, H, W = 8, 3, 1024, 1024
HW = H * W
NA, K = 8, 16
J = 8                      # pixel groups per axis
NB = 4                     # pixel blocks = PE column groups
FREE = 1024                # columns per block tile
N2 = FREE // 2
NCOL = HW // J             # 131072 columns in (c j) n view
NSUP = NCOL // (FREE * NB)  # 32 supergroups

# pair r holds knots (KT[r], KB[r]) in the top/bottom partition halves
KT = [1, 3, 5, 7, 9, 11, 13, 15]
KB = [2, 4, 6, 8, 10, 12, 14, 16]
DVE_PAIRS = [0, 1, 4, 5]
ACT_PAIRS = [2, 6]
GP_PAIRS = [3, 7]

# par columns
P_KV0 = 0          # 0..7: kvec for pair r (DVE/GP tensor_scalar)
P_GSC = 8          # g scale (inv_dx)
P_GBI = 9          # g bias (-mins*inv_dx)
P_BOUT = 10        # drain bias
P_ASC0 = 11        # 11..18: ACT scale per pair
P_ABI0 = 19        # 19..26: ACT bias per pair

_NC_CACHE = {}


def _build_nc():
    nc = bacc.Bacc(None, target_bir_lowering=False, debug=False)
    rawh_t = nc.dram_tensor("rawh", [C * J, NCOL], F16, kind="ExternalInput")
    par_t = nc.dram_tensor("par", [128, 27], F32, kind="ExternalInput")
    wf4_t = nc.dram_tensor("wf4", [128, 128], F16, kind="ExternalInput")
    wlin4_t = nc.dram_tensor("wlin4", [128, C * J], F16, kind="ExternalInput")
    wks_t = nc.dram_tensor("wks", [128, 8 * C * J], F16, kind="ExternalInput")
    out_t = nc.dram_tensor("out", [C * J, NCOL], F16, kind="ExternalOutput")

    Relu = mybir.ActivationFunctionType.Relu
    Ident = mybir.ActivationFunctionType.Identity
    mult = mybir.AluOpType.mult
    add = mybir.AluOpType.add
    mn = mybir.AluOpType.min
    mx = mybir.AluOpType.max
    sub = mybir.AluOpType.subtract

    with tile.TileContext(nc) as tc:
        with (
            tc.tile_pool(name="const", bufs=1) as cpool,
            tc.tile_pool(name="raw", bufs=3) as rawpool,
            tc.tile_pool(name="gg", bufs=3) as gpool,
            tc.tile_pool(name="ff", bufs=10) as fpool,
            tc.tile_pool(name="ob", bufs=3) as obpool,
            tc.tile_pool(name="pf", bufs=4, space="PSUM") as pfpool,
            tc.tile_pool(name="po", bufs=2, space="PSUM") as popool,
        ):
            pT = cpool.tile([128, 27], F32)
            nc.sync.dma_start(out=pT[:], in_=par_t[:])
            wf4 = cpool.tile([128, 128], F16)
            nc.sync.dma_start(out=wf4[:], in_=wf4_t[:])
            wlin4 = cpool.tile([128, C * J], F16)
            nc.sync.dma_start(out=wlin4[:], in_=wlin4_t[:])
            wks = cpool.tile([128, 8 * C * J], F16)
            nc.sync.dma_start(out=wks[:], in_=wks_t[:])

            raw_v = rawh_t.ap()
            out_v = out_t.ap()

            for s in range(NSUP):
                n0 = s * FREE * NB
                rawt = rawpool.tile([128, FREE], F16, tag="raw")
                for b in range(NB):
                    nc.sync.dma_start(
                        out=rawt[32 * b : 32 * b + C * J],
                        in_=raw_v[:, n0 + b * FREE : n0 + (b + 1) * FREE],
                    )
                outp = popool.tile([128, FREE], F32, tag="outp")
                for b in range(NB):
                    sb = slice(32 * b, 32 * b + C * J)
                    # fps (dup halves) then g in fp32 SBUF
                    g = gpool.tile([128, FREE], F32, tag="g")
                    for h in range(2):
                        nn = slice(h * N2, (h + 1) * N2)
                        fps = pfpool.tile([128, N2], F32, tag="fps")
                        nc.tensor.matmul(
                            fps[:],
                            wf4[sb],
                            rawt[sb, nn],
                            start=True,
                            stop=True,
                            tile_position=(32 * b, 0),
                        )
                        nc.scalar.activation(
                            g[:, nn],
                            fps[:],
                            Ident,
                            bias=pT[:, P_GBI : P_GBI + 1],
                            scale=pT[:, P_GSC : P_GSC + 1],
                        )
                    # linear + constant-free matmul opens the accum group
                    for h in range(2):
                        nn = slice(h * N2, (h + 1) * N2)
                        nc.tensor.matmul(
                            outp[sb, nn],
                            wlin4[sb],
                            rawt[sb, nn],
                            start=True,
                            stop=False,
                            tile_position=(32 * b, 32 * b),
                        )
                    for r in range(8):
                        ft = fpool.tile([128, FREE], F16, tag="ft")
                        if r in ACT_PAIRS:
                            nc.scalar.activation(
                                ft[:],
                                g[:],
                                Relu,
                                bias=pT[:, P_ABI0 + r : P_ABI0 + r + 1],
                                scale=pT[:, P_ASC0 + r : P_ASC0 + r + 1],
                            )
                        else:
                            eng = nc.vector if r in DVE_PAIRS else nc.gpsimd
                            eng.tensor_scalar(
                                out=ft[:],
                                in0=g[:],
                                scalar1=pT[:, P_KV0 + r : P_KV0 + r + 1],
                                scalar2=0.0,
                                op0=sub,
                                op1=mn if r < 4 else mx,
                            )
                        wk = wks[:, r * C * J : (r + 1) * C * J]
                        for h in range(2):
                            nn = slice(h * N2, (h + 1) * N2)
                            nc.tensor.matmul(
                                outp[sb, nn],
                                wk,
                                ft[:, nn],
                                start=False,
                                stop=(r == 7),
                                tile_position=(0, 32 * b),
                            )
                ob = obpool.tile([128, FREE], F16, tag="ob")
                nc.vector.tensor_scalar(
                    out=ob[:],
                    in0=outp[:],
                    scalar1=1.0,
                    scalar2=pT[:, P_BOUT : P_BOUT + 1],
                    op0=mult,
                    op1=add,
                )
                for b in range(NB):
                    nc.sync.dma_start(
                        out=out_v[:, n0 + b * FREE : n0 + (b + 1) * FREE],
                        in_=ob[32 * b : 32 * b + C * J],
                    )
    nc.compile()
    return nc


def _host_params(raw, ys, A):
    in_maps = []
    for b in range(B):
        Ab = A[b].astype(np.float64)
        mins = np.minimum(Ab, 0).sum(0)
        maxs = np.maximum(Ab, 0).sum(0)
        pinv = np.linalg.pinv(Ab)  # [8, 3]
        dx = (maxs + EPS - mins) / (K + 1)
        inv_dx = 1.0 / dx
        bias_g = -mins * inv_dx
        Y = np.concatenate([mins[:, None], ys[b].astype(np.float64), maxs[:, None]], 1)
        dY = np.diff(Y, 1)                                   # [8, 17]
        d = np.concatenate([dY[:, :1], np.diff(dY, axis=1)], 1)  # [8, 17]
        lin = dY[:, 0] + d[:, 1:9].sum(1)                    # two-sided fold
        koff = (d[:, 1:9] * np.arange(1, 9)).sum(1)
        const_c = pinv.T @ (Y[:, 0] - koff + lin * bias_g)   # [3]

        par = np.zeros((128, 27), np.float32)
        # partition p = h*64 + a*8 + j
        aidx = (np.arange(128) // J) % NA
        hidx = np.arange(128) // 64
        for r in range(8):
            kv = np.where(hidx == 0, KT[r], KB[r]).astype(np.float64)
            par[:, P_KV0 + r] = kv
            sgn = -1.0 if r < 4 else 1.0
            par[:, P_ASC0 + r] = sgn
            par[:, P_ABI0 + r] = -sgn * kv
        par[:, P_GSC] = inv_dx[aidx]
        par[:, P_GBI] = bias_g[aidx]
        cidx = (np.arange(128) % 32) // J
        bvec = const_c[np.minimum(cidx, 2)]
        par[:, P_BOUT] = bvec

        A16 = Ab.astype(np.float16).astype(np.float64)
        wf = np.zeros((C * J, 128), np.float64)   # (c j) -> (h a j)
        for c in range(C):
            for j in range(J):
                for h in range(2):
                    for a in range(NA):
                        wf[c * J + j, h * 64 + a * J + j] = A16[c, a]
        wf4 = np.zeros((128, 128), np.float16)
        wlin = np.einsum("ac,a,ka->kc", pinv, lin * inv_dx, Ab)  # [c_in=3, 3]
        wlin4 = np.zeros((128, C * J), np.float16)
        wl = np.zeros((C * J, C * J), np.float64)
        for j in range(J):
            wl[j::J, j::J] = wlin  # rows c_in, cols c_out
        for bk in range(NB):
            wf4[32 * bk : 32 * bk + C * J] = wf.astype(np.float16)
            wlin4[32 * bk : 32 * bk + C * J] = wl.astype(np.float16)

        wks = np.zeros((128, 8 * C * J), np.float16)
        for r in range(8):
            for h, k in ((0, KT[r]), (1, KB[r])):
                if k <= 8:
                    sgn = 1.0 if r in ACT_PAIRS else -1.0
                else:
                    sgn = 1.0
                for a in range(NA):
                    for j in range(J):
                        for c in range(C):
                            wks[h * 64 + a * J + j, r * C * J + c * J + j] = (
                                sgn * d[a, k] * pinv[a, c]
                            )

        rb = raw[b].reshape(C, J, NCOL).reshape(C * J, NCOL)
        in_maps.append(
            {
                "rawh": rb.astype(np.float16),
                "par": par,
                "wf4": wf4,
                "wlin4": wlin4,
                "wks": wks,
            }
        )
    return in_maps


def kernel(raw, ys, A):
    raw = np.asarray(raw, np.float32)
    ys = np.asarray(ys, np.float32)
    A = np.asarray(A, np.float32)
    if "nc" not in _NC_CACHE:
        _NC_CACHE["nc"] = _build_nc()
    nc = _NC_CACHE["nc"]
    in_maps = _host_params(raw, ys, A)
    res = run_bass_kernel_spmd(nc, in_maps, core_ids=list(range(B)))
    out = np.stack(
        [
            res.results[b]["out"]
            .astype(np.float32)
            .reshape(C, J, NCOL)
            .reshape(C, H, W)
            for b in range(B)
        ]
    )
    return out


# revision 4
# speedup vs baseline: 5.0785x; 5.0785x over previous
"""Trainium2 Bass kernel for nn_AxisSimplestSpline — relu-basis rewrite.

Math (per batch b, axis a):  g = (f - mins)/dx in [0,17),  f = A^T raw.
  est_a(g) = Y0 + lin_a*g + sum_{k=1..16} d_k * basis_k(g)
with d_k the slope-diffs of the PWL and a *two-sided* relu basis to keep
fp16 feature magnitudes <= 8:
  k in 1..8 : basis = min(g-k, 0)   (weight -d_k)   [DVE/GPSIMD]
              or relu(k-g)          (weight +d_k)   [ACT]
  k in 9..16: basis = relu(g-k)     (weight +d_k)
The linear remainder d_k*(g-k) of the min-side knots folds into lin_a.

out[c] = sum_a pinv[a,c] est_a: the per-knot weights fuse d_k*pinv, the
linear term lin_a*g becomes one extra matmul on raw itself, and the
constant goes into the drain bias.  Engine split per block: ACT builds
g (fp32, SBUF) + 2 knot-pairs, DVE 4 pairs + PSUM drain, GPSIMD 2 pairs.
Each pair of knots shares one matmul (features stacked 2x(8a x 8j) =
K=128, M=24); 4 pixel blocks land on distinct PE column-groups via
tile_position so their matmuls overlap in the array.
"""

import sys

sys.path.insert(0, "/opt/trn_rl_repo")

import numpy as np

import concourse.bacc as bacc
import concourse.mybir as mybir
import concourse.tile as tile
from concourse.bass_utils import run_bass_kernel_spmd

F32 = mybir.dt.float32
F16 = mybir.dt.float16
EPS = 1e-4
B, C, H, W = 8, 3, 1024, 1024
HW = H * W
NA, K = 8, 16
J = 8                      # pixel groups per axis
NB = 4                     # pixel blocks = PE column groups
FREE = 1024                # columns per block tile
N2 = FREE // 2
NCOL = HW // J             # 131072 columns in (c j) n view
NSUP = NCOL // (FREE * NB)  # 32 supergroups

# pair r holds knots (KT[r], KB[r]) in the top/bottom partition halves
KT = [1, 3, 5, 7, 9, 11, 13, 15]
KB = [2, 4, 6, 8, 10, 12, 14, 16]
DVE_PAIRS = [0, 1, 3, 4, 5, 7]
ACT_PAIRS = [2, 6]
GP_PAIRS = []

# par columns
P_KV0 = 0          # 0..7: kvec for pair r (DVE/GP tensor_scalar)
P_GSC = 8          # g scale (inv_dx)
P_GBI = 9          # g bias (-mins*inv_dx)
P_BOUT = 10        # drain bias
P_ASC0 = 11        # 11..18: ACT scale per pair
P_ABI0 = 19        # 19..26: ACT bias per pair

_NC_CACHE = {}


def _build_nc():
    nc = bacc.Bacc(None, target_bir_lowering=False, debug=False)
    rawh_t = nc.dram_tensor("rawh", [C * J, NCOL], F16, kind="ExternalInput")
    par_t = nc.dram_tensor("par", [128, 27], F32, kind="ExternalInput")
    wf4_t = nc.dram_tensor("wf4", [128, 128], F16, kind="ExternalInput")
    wlin4_t = nc.dram_tensor("wlin4", [128, C * J], F16, kind="ExternalInput")
    wks_t = nc.dram_tensor("wks", [128, 8 * C * J], F16, kind="ExternalInput")
    out_t = nc.dram_tensor("out", [C * J, NCOL], F16, kind="ExternalOutput")

    Relu = mybir.ActivationFunctionType.Relu
    Ident = mybir.ActivationFunctionType.Identity
    mult = mybir.AluOpType.mult
    add = mybir.AluOpType.add
    mn = mybir.AluOpType.min
    mx = mybir.AluOpType.max
    sub = mybir.AluOpType.subtract

    with tile.TileContext(nc) as tc:
        with (
            tc.tile_pool(name="const", bufs=1) as cpool,
            tc.tile_pool(name="raw", bufs=3) as rawpool,
            tc.tile_pool(name="gg", bufs=3) as gpool,
            tc.tile_pool(name="ff", bufs=10) as fpool,
            tc.tile_pool(name="ob", bufs=3) as obpool,
            tc.tile_pool(name="pf", bufs=4, space="PSUM") as pfpool,
            tc.tile_pool(name="po", bufs=2, space="PSUM") as popool,
        ):
            pT = cpool.tile([128, 27], F32)
            nc.sync.dma_start(out=pT[:], in_=par_t[:])
            wf4 = cpool.tile([128, 128], F16)
            nc.sync.dma_start(out=wf4[:], in_=wf4_t[:])
            wlin4 = cpool.tile([128, C * J], F16)
            nc.sync.dma_start(out=wlin4[:], in_=wlin4_t[:])
            wks = cpool.tile([128, 8 * C * J], F16)
            nc.sync.dma_start(out=wks[:], in_=wks_t[:])

            raw_v = rawh_t.ap()
            out_v = out_t.ap()

            for s in range(NSUP):
                n0 = s * FREE * NB
                rawt = rawpool.tile([128, FREE], F16, tag="raw")
                for b in range(NB):
                    nc.sync.dma_start(
                        out=rawt[32 * b : 32 * b + C * J],
                        in_=raw_v[:, n0 + b * FREE : n0 + (b + 1) * FREE],
                    )
                outp = popool.tile([128, FREE], F32, tag="outp")
                for b in range(NB):
                    sb = slice(32 * b, 32 * b + C * J)
                    # fps (dup halves) then g in fp32 SBUF
                    g = gpool.tile([128, FREE], F32, tag="g")
                    for h in range(2):
                        nn = slice(h * N2, (h + 1) * N2)
                        fps = pfpool.tile([128, N2], F32, tag="fps")
                        nc.tensor.matmul(
                            fps[:],
                            wf4[sb],
                            rawt[sb, nn],
                            start=True,
                            stop=True,
                            tile_position=(32 * b, 0),
                        )
                        nc.scalar.activation(
                            g[:, nn],
                            fps[:],
                            Ident,
                            bias=pT[:, P_GBI : P_GBI + 1],
                            scale=pT[:, P_GSC : P_GSC + 1],
                        )
                    # linear + constant-free matmul opens the accum group
                    for h in range(2):
                        nn = slice(h * N2, (h + 1) * N2)
                        nc.tensor.matmul(
                            outp[sb, nn],
                            wlin4[sb],
                            rawt[sb, nn],
                            start=True,
                            stop=False,
                            tile_position=(32 * b, 32 * b),
                        )
                    for r in range(8):
                        ft = fpool.tile([128, FREE], F16, tag="ft")
                        if r in ACT_PAIRS:
                            nc.scalar.activation(
                                ft[:],
                                g[:],
                                Relu,
                                bias=pT[:, P_ABI0 + r : P_ABI0 + r + 1],
                                scale=pT[:, P_ASC0 + r : P_ASC0 + r + 1],
                            )
                        else:
                            eng = nc.vector if r in DVE_PAIRS else nc.gpsimd
                            eng.tensor_scalar(
                                out=ft[:],
                                in0=g[:],
                                scalar1=pT[:, P_KV0 + r : P_KV0 + r + 1],
                                scalar2=0.0,
                                op0=sub,
                                op1=mn if r < 4 else mx,
                            )
                        wk = wks[:, r * C * J : (r + 1) * C * J]
                        for h in range(2):
                            nn = slice(h * N2, (h + 1) * N2)
                            nc.tensor.matmul(
                                outp[sb, nn],
                                wk,
                                ft[:, nn],
                                start=False,
                                stop=(r == 7),
                                tile_position=(0, 32 * b),
                            )
                ob = obpool.tile([128, FREE], F16, tag="ob")
                nc.vector.tensor_scalar(
                    out=ob[:],
                    in0=outp[:],
                    scalar1=1.0,
                    scalar2=pT[:, P_BOUT : P_BOUT + 1],
                    op0=mult,
                    op1=add,
                )
                for b in range(NB):
                    nc.sync.dma_start(
                        out=out_v[:, n0 + b * FREE : n0 + (b + 1) * FREE],
                        in_=ob[32 * b : 32 * b + C * J],
                    )
    nc.compile()
    return nc


def _host_params(raw, ys, A):
    in_maps = []
    for b in range(B):
        Ab = A[b].astype(np.float64)
        mins = np.minimum(Ab, 0).sum(0)
        maxs = np.maximum(Ab, 0).sum(0)
        pinv = np.linalg.pinv(Ab)  # [8, 3]
        dx = (maxs + EPS - mins) / (K + 1)
        inv_dx = 1.0 / dx
        bias_g = -mins * inv_dx
        Y = np.concatenate([mins[:, None], ys[b].astype(np.float64), maxs[:, None]], 1)
        dY = np.diff(Y, 1)                                   # [8, 17]
        d = np.concatenate([dY[:, :1], np.diff(dY, axis=1)], 1)  # [8, 17]
        lin = dY[:, 0] + d[:, 1:9].sum(1)                    # two-sided fold
        koff = (d[:, 1:9] * np.arange(1, 9)).sum(1)
        const_c = pinv.T @ (Y[:, 0] - koff + lin * bias_g)   # [3]

        par = np.zeros((128, 27), np.float32)
        # partition p = h*64 + a*8 + j
        aidx = (np.arange(128) // J) % NA
        hidx = np.arange(128) // 64
        for r in range(8):
            kv = np.where(hidx == 0, KT[r], KB[r]).astype(np.float64)
            par[:, P_KV0 + r] = kv
            sgn = -1.0 if r < 4 else 1.0
            par[:, P_ASC0 + r] = sgn
            par[:, P_ABI0 + r] = -sgn * kv
        par[:, P_GSC] = inv_dx[aidx]
        par[:, P_GBI] = bias_g[aidx]
        cidx = (np.arange(128) % 32) // J
        bvec = const_c[np.minimum(cidx, 2)]
        par[:, P_BOUT] = bvec

        A16 = Ab.astype(np.float16).astype(np.float64)
        wf = np.zeros((C * J, 128), np.float64)   # (c j) -> (h a j)
        for c in range(C):
            for j in range(J):
                for h in range(2):
                    for a in range(NA):
                        wf[c * J + j, h * 64 + a * J + j] = A16[c, a]
        wf4 = np.zeros((128, 128), np.float16)
        wlin = np.einsum("ac,a,ka->kc", pinv, lin * inv_dx, Ab)  # [c_in=3, 3]
        wlin4 = np.zeros((128, C * J), np.float16)
        wl = np.zeros((C * J, C * J), np.float64)
        for j in range(J):
            wl[j::J, j::J] = wlin  # rows c_in, cols c_out
        for bk in range(NB):
            wf4[32 * bk : 32 * bk + C * J] = wf.astype(np.float16)
            wlin4[32 * bk : 32 * bk + C * J] = wl.astype(np.float16)

        wks = np.zeros((128, 8 * C * J), np.float16)
        for r in range(8):
            for h, k in ((0, KT[r]), (1, KB[r])):
                if k <= 8:
                    sgn = 1.0 if r in ACT_PAIRS else -1.0
                else:
                    sgn = 1.0
                for a in range(NA):
                    for j in range(J):
                        for c in range(C):
                            wks[h * 64 + a * J + j, r * C * J + c * J + j] = (
                                sgn * d[a, k] * pinv[a, c]
                            )

        rb = raw[b].reshape(C, J, NCOL).reshape(C * J, NCOL)
        in_maps.append(
            {
                "rawh": rb.astype(np.float16),
                "par": par,
                "wf4": wf4,
                "wlin4": wlin4,
                "wks": wks,
            }
        )
    return in_maps


def kernel(raw, ys, A):
    raw = np.asarray(raw, np.float32)
    ys = np.asarray(ys, np.float32)
    A = np.asarray(A, np.float32)
    if "nc" not in _NC_CACHE:
        _NC_CACHE["nc"] = _build_nc()
    nc = _NC_CACHE["nc"]
    in_maps = _host_params(raw, ys, A)
    res = run_bass_kernel_spmd(nc, in_maps, core_ids=list(range(B)))
    out = np.stack(
        [
            res.results[b]["out"]
            .astype(np.float32)
            .reshape(C, J, NCOL)
            .reshape(C, H, W)
            for b in range(B)
        ]
    )
    return out


# revision 5
# speedup vs baseline: 7.5987x; 1.4963x over previous
"""Trainium2 Bass kernel for nn_AxisSimplestSpline — relu-basis, fp16 4x DVE.

Math (per batch b, axis a):  g = (f - mins)/dx in [0,17),  f = A^T raw.
  est_a(g) = Y0 + lin_a*g + sum_{k=1..16} d_k * basis_k(g)
with d_k the PWL slope-diffs and a two-sided relu basis evaluated through
half-range fp16 tensors (magnitude <= 8.5, so fp16 keeps ~2^-12 accuracy):
  hA = relu(8.5 - g), hB = relu(g - 8.5)            (one ACT pass, stacked)
  k in 1..8 : relu(k - g)  = relu(hA - (8.5-k))     (weight +d_k; the
              linear remainder d_k*(g-k) folds into lin_a)
  k in 9..16: relu(g - k)  = relu(hB - (k-8.5))     (weight +d_k)
All 16 knot features are fp16 tensor_scalar (sub,max) on DVE in 4x mode,
two knots per pass (hA in partitions 0:64, hB in 64:128).  The linear
term lin_a*g is one extra matmul on raw; the constant rides the drain
bias.  Knot matmuls are K=128/M=24 with the four pixel blocks on
distinct PE column groups (tile_position) issued r-major so they
co-execute in the array.  GPSIMD is left idle: its SBUF port is shared
with DVE and concurrent streaming poisons both (measured 10x).
"""

import sys

sys.path.insert(0, "/opt/trn_rl_repo")

import numpy as np

import concourse.bacc as bacc
import concourse.mybir as mybir
import concourse.tile as tile
from concourse.bass_utils import run_bass_kernel_spmd

F32 = mybir.dt.float32
F16 = mybir.dt.float16
EPS = 1e-4
B, C, H, W = 8, 3, 1024, 1024
HW = H * W
NA, K = 8, 16
J = 8                      # pixel groups per axis
NB = 4                     # pixel blocks = PE column groups
FREE = 1024                # columns per block tile
N2 = FREE // 2
NCOL = HW // J             # 131072 columns in (c j) n view
NSUP = NCOL // (FREE * NB)  # 32 supergroups

# pair r: top half = min-side knot r+1 (via hA), bottom = max-side knot r+9
# (via hB); both are (x - c) then relu with c per partition.
ACT_PAIRS = []             # pairs computed on ACT (from hstack); rest on DVE

# par columns
P_KV0 = 0          # 0..7: c-vec for pair r
P_HSC = 8          # hstack ACT scale (-inv_dx top / +inv_dx bottom)
P_HBI = 9          # hstack ACT bias (8.5-bias_g top / bias_g-8.5 bottom)
P_BOUT = 10        # drain bias

_NC_CACHE = {}


def _build_nc():
    nc = bacc.Bacc(None, target_bir_lowering=False, debug=False)
    rawh_t = nc.dram_tensor("rawh", [C * J, NCOL], F16, kind="ExternalInput")
    par_t = nc.dram_tensor("par", [128, 11], F32, kind="ExternalInput")
    wf4_t = nc.dram_tensor("wf4", [128, 128], F16, kind="ExternalInput")
    wlin4_t = nc.dram_tensor("wlin4", [128, C * J], F16, kind="ExternalInput")
    wks_t = nc.dram_tensor("wks", [128, 8 * C * J], F16, kind="ExternalInput")
    out_t = nc.dram_tensor("out", [C * J, NCOL], F16, kind="ExternalOutput")

    Relu = mybir.ActivationFunctionType.Relu
    mult = mybir.AluOpType.mult
    add = mybir.AluOpType.add
    mx = mybir.AluOpType.max
    sub = mybir.AluOpType.subtract

    with tile.TileContext(nc) as tc:
        with (
            tc.tile_pool(name="const", bufs=1) as cpool,
            tc.tile_pool(name="raw", bufs=3) as rawpool,
            tc.tile_pool(name="hh", bufs=6) as hpool,
            tc.tile_pool(name="ff", bufs=10) as fpool,
            tc.tile_pool(name="ob", bufs=3) as obpool,
            tc.tile_pool(name="pf", bufs=2, space="PSUM") as pfpool,
            tc.tile_pool(name="po", bufs=2, space="PSUM") as popool,
        ):
            pT = cpool.tile([128, 11], F32)
            nc.sync.dma_start(out=pT[:], in_=par_t[:])
            wf4 = cpool.tile([128, 128], F16)
            nc.sync.dma_start(out=wf4[:], in_=wf4_t[:])
            wlin4 = cpool.tile([128, C * J], F16)
            nc.sync.dma_start(out=wlin4[:], in_=wlin4_t[:])
            wks = cpool.tile([128, 8 * C * J], F16)
            nc.sync.dma_start(out=wks[:], in_=wks_t[:])

            raw_v = rawh_t.ap()
            out_v = out_t.ap()

            for s in range(NSUP):
                n0 = s * FREE * NB
                rawt = rawpool.tile([128, FREE], F16, tag="raw")
                for b in range(NB):
                    nc.sync.dma_start(
                        out=rawt[32 * b : 32 * b + C * J],
                        in_=raw_v[:, n0 + b * FREE : n0 + (b + 1) * FREE],
                    )
                outp = popool.tile([128, FREE], F32, tag="outp")
                hs = []
                for b in range(NB):
                    sb = slice(32 * b, 32 * b + C * J)
                    fps = pfpool.tile([128, FREE], F32, tag="fps")
                    for h in range(2):
                        nn = slice(h * N2, (h + 1) * N2)
                        nc.tensor.matmul(
                            fps[:, nn],
                            wf4[sb],
                            rawt[sb, nn],
                            start=True,
                            stop=True,
                            tile_position=(32 * b, 0),
                        )
                    hst = hpool.tile([128, FREE], F16, tag="h")
                    nc.scalar.activation(
                        hst[:],
                        fps[:],
                        Relu,
                        bias=pT[:, P_HBI : P_HBI + 1],
                        scale=pT[:, P_HSC : P_HSC + 1],
                    )
                    hs.append(hst)
                for b in range(NB):
                    sb = slice(32 * b, 32 * b + C * J)
                    for h in range(2):
                        nn = slice(h * N2, (h + 1) * N2)
                        nc.tensor.matmul(
                            outp[sb, nn],
                            wlin4[sb],
                            rawt[sb, nn],
                            start=True,
                            stop=False,
                            tile_position=(32 * b, 32 * b),
                        )
                for r in range(8):
                    for b in range(NB):
                        sb = slice(32 * b, 32 * b + C * J)
                        ft = fpool.tile([128, FREE], F16, tag="ft")
                        if r in ACT_PAIRS:
                            nc.scalar.activation(
                                ft[:],
                                hs[b][:],
                                Relu,
                                bias=pT[:, P_KV0 + r : P_KV0 + r + 1],
                                scale=1.0,
                            )
                        else:
                            nc.vector.tensor_scalar(
                                out=ft[:],
                                in0=hs[b][:],
                                scalar1=pT[:, P_KV0 + r : P_KV0 + r + 1],
                                scalar2=0.0,
                                op0=sub,
                                op1=mx,
                            )
                        wk = wks[:, r * C * J : (r + 1) * C * J]
                        for h in range(2):
                            nn = slice(h * N2, (h + 1) * N2)
                            nc.tensor.matmul(
                                outp[sb, nn],
                                wk,
                                ft[:, nn],
                                start=False,
                                stop=(r == 7),
                                tile_position=(0, 32 * b),
                            )
                ob = obpool.tile([128, FREE], F16, tag="ob")
                nc.vector.tensor_scalar(
                    out=ob[:],
                    in0=outp[:],
                    scalar1=1.0,
                    scalar2=pT[:, P_BOUT : P_BOUT + 1],
                    op0=mult,
                    op1=add,
                )
                for b in range(NB):
                    nc.sync.dma_start(
                        out=out_v[:, n0 + b * FREE : n0 + (b + 1) * FREE],
                        in_=ob[32 * b : 32 * b + C * J],
                    )
    nc.compile()
    return nc


def _host_params(raw, ys, A):
    in_maps = []
    for b in range(B):
        Ab = A[b].astype(np.float64)
        mins = np.minimum(Ab, 0).sum(0)
        maxs = np.maximum(Ab, 0).sum(0)
        pinv = np.linalg.pinv(Ab)  # [8, 3]
        dx = (maxs + EPS - mins) / (K + 1)
        inv_dx = 1.0 / dx
        bias_g = -mins * inv_dx
        Y = np.concatenate([mins[:, None], ys[b].astype(np.float64), maxs[:, None]], 1)
        dY = np.diff(Y, 1)                                   # [8, 17]
        d = np.concatenate([dY[:, :1], np.diff(dY, axis=1)], 1)  # [8, 17]
        lin = dY[:, 0] + d[:, 1:9].sum(1)                    # two-sided fold
        koff = (d[:, 1:9] * np.arange(1, 9)).sum(1)
        const_c = pinv.T @ (Y[:, 0] - koff + lin * bias_g)   # [3]

        par = np.zeros((128, 11), np.float32)
        # partition p = h*64 + a*8 + j
        aidx = (np.arange(128) // J) % NA
        hidx = np.arange(128) // 64
        for r in range(8):
            # feature = relu(h - c): top c = 8.5-(r+1), bottom c = (r+9)-8.5
            cv = np.where(hidx == 0, 8.5 - (r + 1), (r + 9) - 8.5)
            if r in ACT_PAIRS:
                par[:, P_KV0 + r] = -cv  # ACT bias
            else:
                par[:, P_KV0 + r] = cv   # DVE subtract
        sgn = np.where(hidx == 0, -1.0, 1.0)
        par[:, P_HSC] = sgn * inv_dx[aidx]
        par[:, P_HBI] = sgn * (bias_g[aidx] - 8.5)
        cidx = (np.arange(128) % 32) // J
        par[:, P_BOUT] = const_c[np.minimum(cidx, 2)]

        A16 = Ab.astype(np.float16).astype(np.float64)
        wf = np.zeros((C * J, 128), np.float64)   # (c j) -> (h a j)
        for c in range(C):
            for j in range(J):
                for h in range(2):
                    for a in range(NA):
                        wf[c * J + j, h * 64 + a * J + j] = A16[c, a]
        wlin = np.einsum("ac,a,ka->kc", pinv, lin * inv_dx, Ab)  # [3, 3]
        wl = np.zeros((C * J, C * J), np.float64)
        for j in range(J):
            wl[j::J, j::J] = wlin
        wf4 = np.zeros((128, 128), np.float16)
        wlin4 = np.zeros((128, C * J), np.float16)
        for bk in range(NB):
            wf4[32 * bk : 32 * bk + C * J] = wf.astype(np.float16)
            wlin4[32 * bk : 32 * bk + C * J] = wl.astype(np.float16)

        wks = np.zeros((128, 8 * C * J), np.float16)
        for r in range(8):
            for h, k in ((0, r + 1), (1, r + 9)):
                for a in range(NA):
                    w_ac = d[a, k] * pinv[a]  # [3]
                    for j in range(J):
                        for c in range(C):
                            wks[h * 64 + a * J + j, r * C * J + c * J + j] = w_ac[c]

        rb = raw[b].reshape(C, J, NCOL).reshape(C * J, NCOL)
        in_maps.append(
            {
                "rawh": rb.astype(np.float16),
                "par": par,
                "wf4": wf4,
                "wlin4": wlin4,
                "wks": wks,
            }
        )
    return in_maps


def kernel(raw, ys, A):
    raw = np.asarray(raw, np.float32)
    ys = np.asarray(ys, np.float32)
    A = np.asarray(A, np.float32)
    if "nc" not in _NC_CACHE:
        _NC_CACHE["nc"] = _build_nc()
    nc = _NC_CACHE["nc"]
    in_maps = _host_params(raw, ys, A)
    res = run_bass_kernel_spmd(nc, in_maps, core_ids=list(range(B)))
    out = np.stack(
        [
            res.results[b]["out"]
            .astype(np.float32)
            .reshape(C, J, NCOL)
            .reshape(C, H, W)
            for b in range(B)
        ]
    )
    return out


# revision 6
# speedup vs baseline: 7.8784x; 1.0368x over previous
"""Trainium2 Bass kernel for nn_AxisSimplestSpline — relu-basis, fp16 4x DVE.

Math (per batch b, axis a):  g = (f - mins)/dx in [0,17),  f = A^T raw.
  est_a(g) = Y0 + lin_a*g + sum_{k=1..16} d_k * basis_k(g)
with d_k the PWL slope-diffs and a two-sided relu basis evaluated through
half-range fp16 tensors (magnitude <= 8.5, so fp16 keeps ~2^-12 accuracy):
  hA = relu(8.5 - g), hB = relu(g - 8.5)            (one ACT pass, stacked)
  k in 1..8 : relu(k - g)  = relu(hA - (8.5-k))     (weight +d_k; the
              linear remainder d_k*(g-k) folds into lin_a)
  k in 9..16: relu(g - k)  = relu(hB - (k-8.5))     (weight +d_k)
All 16 knot features are fp16 tensor_scalar (sub,max) on DVE in 4x mode,
two knots per pass (hA in partitions 0:64, hB in 64:128).  The linear
term lin_a*g is one extra matmul on raw; the constant rides the drain
bias.  Knot matmuls are K=128/M=24 with the four pixel blocks on
distinct PE column groups (tile_position) issued r-major so they
co-execute in the array.  GPSIMD is left idle: its SBUF port is shared
with DVE and concurrent streaming poisons both (measured 10x).
"""

import sys

sys.path.insert(0, "/opt/trn_rl_repo")

import numpy as np

import concourse.bacc as bacc
import concourse.mybir as mybir
import concourse.tile as tile
from concourse.bass_utils import run_bass_kernel_spmd

F32 = mybir.dt.float32
F16 = mybir.dt.float16
EPS = 1e-4
B, C, H, W = 8, 3, 1024, 1024
HW = H * W
NA, K = 8, 16
J = 8                      # pixel groups per axis
NB = 4                     # pixel blocks = PE column groups
FREE = 1024                # columns per block tile
N2 = FREE // 2
NCOL = HW // J             # 131072 columns in (c j) n view
NSUP = NCOL // (FREE * NB)  # 32 supergroups

# pair r: top half = min-side knot r+1 (via hA), bottom = max-side knot r+9
# (via hB); both are (x - c) then relu with c per partition.
ACT_PAIRS = []             # pairs computed on ACT (from hstack); rest on DVE

# par columns
P_KV0 = 0          # 0..7: c-vec for pair r
P_HSC = 8          # hstack ACT scale (-inv_dx top / +inv_dx bottom)
P_HBI = 9          # hstack ACT bias (8.5-bias_g top / bias_g-8.5 bottom)
P_BOUT = 10        # drain bias

_NC_CACHE = {}


def _build_nc():
    nc = bacc.Bacc(None, target_bir_lowering=False, debug=False)
    rawh_t = nc.dram_tensor("rawh", [C * J, NCOL], F16, kind="ExternalInput")
    par_t = nc.dram_tensor("par", [128, 11], F32, kind="ExternalInput")
    wf4_t = nc.dram_tensor("wf4", [128, 128], F16, kind="ExternalInput")
    wlin4_t = nc.dram_tensor("wlin4", [128, C * J], F16, kind="ExternalInput")
    wks_t = nc.dram_tensor("wks", [128, 8 * C * J], F16, kind="ExternalInput")
    out_t = nc.dram_tensor("out", [C * J, NCOL], F16, kind="ExternalOutput")

    Relu = mybir.ActivationFunctionType.Relu
    mult = mybir.AluOpType.mult
    add = mybir.AluOpType.add
    mx = mybir.AluOpType.max
    sub = mybir.AluOpType.subtract

    with tile.TileContext(nc) as tc:
        with (
            tc.tile_pool(name="const", bufs=1) as cpool,
            tc.tile_pool(name="raw", bufs=3) as rawpool,
            tc.tile_pool(name="hh", bufs=6) as hpool,
            tc.tile_pool(name="ff", bufs=10) as fpool,
            tc.tile_pool(name="ob", bufs=3) as obpool,
            tc.tile_pool(name="pf", bufs=2, space="PSUM") as pfpool,
            tc.tile_pool(name="po", bufs=2, space="PSUM") as popool,
        ):
            pT = cpool.tile([128, 11], F32)
            nc.sync.dma_start(out=pT[:], in_=par_t[:])
            wf4 = cpool.tile([128, 128], F16)
            nc.sync.dma_start(out=wf4[:], in_=wf4_t[:])
            wlin4 = cpool.tile([128, C * J], F16)
            nc.sync.dma_start(out=wlin4[:], in_=wlin4_t[:])
            wks = cpool.tile([128, 8 * C * J], F16)
            nc.sync.dma_start(out=wks[:], in_=wks_t[:])

            raw_v = rawh_t.ap()
            out_v = out_t.ap()

            for s in range(NSUP):
                n0 = s * FREE * NB
                rawt = rawpool.tile([128, FREE], F16, tag="raw")
                for b in range(NB):
                    nc.sync.dma_start(
                        out=rawt[32 * b : 32 * b + C * J],
                        in_=raw_v[:, n0 + b * FREE : n0 + (b + 1) * FREE],
                    )
                outp = popool.tile([128, FREE], F32, tag="outp")
                hs = []
                for b in range(NB):
                    sb = slice(32 * b, 32 * b + C * J)
                    fps = pfpool.tile([128, FREE], F32, tag="fps")
                    for h in range(2):
                        nn = slice(h * N2, (h + 1) * N2)
                        nc.tensor.matmul(
                            fps[:, nn],
                            wf4[sb],
                            rawt[sb, nn],
                            start=True,
                            stop=True,
                            tile_position=(32 * b, 0),
                        )
                    hst = hpool.tile([128, FREE], F16, tag="h")
                    nc.scalar.activation(
                        hst[:],
                        fps[:],
                        Relu,
                        bias=pT[:, P_HBI : P_HBI + 1],
                        scale=pT[:, P_HSC : P_HSC + 1],
                    )
                    hs.append(hst)
                for b in range(NB):
                    sb = slice(32 * b, 32 * b + C * J)
                    for h in range(2):
                        nn = slice(h * N2, (h + 1) * N2)
                        nc.tensor.matmul(
                            outp[sb, nn],
                            wlin4[sb],
                            rawt[sb, nn],
                            start=True,
                            stop=False,
                            tile_position=(32 * b, 32 * b),
                        )
                for r in range(8):
                    fts = []
                    for b in range(NB):
                        ft = fpool.tile([128, FREE], F16, tag="ft")
                        if r in ACT_PAIRS:
                            nc.scalar.activation(
                                ft[:],
                                hs[b][:],
                                Relu,
                                bias=pT[:, P_KV0 + r : P_KV0 + r + 1],
                                scale=1.0,
                            )
                        else:
                            nc.vector.tensor_scalar(
                                out=ft[:],
                                in0=hs[b][:],
                                scalar1=pT[:, P_KV0 + r : P_KV0 + r + 1],
                                scalar2=0.0,
                                op0=sub,
                                op1=mx,
                            )
                        fts.append(ft)
                    wk = wks[:, r * C * J : (r + 1) * C * J]
                    for h in range(2):
                        nn = slice(h * N2, (h + 1) * N2)
                        for b in range(NB):
                            sb = slice(32 * b, 32 * b + C * J)
                            nc.tensor.matmul(
                                outp[sb, nn],
                                wk,
                                fts[b][:, nn],
                                start=False,
                                stop=(r == 7),
                                tile_position=(0, 32 * b),
                            )
                ob = obpool.tile([128, FREE], F16, tag="ob")
                nc.vector.tensor_scalar(
                    out=ob[:],
                    in0=outp[:],
                    scalar1=1.0,
                    scalar2=pT[:, P_BOUT : P_BOUT + 1],
                    op0=mult,
                    op1=add,
                )
                for b in range(NB):
                    nc.sync.dma_start(
                        out=out_v[:, n0 + b * FREE : n0 + (b + 1) * FREE],
                        in_=ob[32 * b : 32 * b + C * J],
                    )
    nc.compile()
    return nc


def _host_params(raw, ys, A):
    in_maps = []
    for b in range(B):
        Ab = A[b].astype(np.float64)
        mins = np.minimum(Ab, 0).sum(0)
        maxs = np.maximum(Ab, 0).sum(0)
        pinv = np.linalg.pinv(Ab)  # [8, 3]
        dx = (maxs + EPS - mins) / (K + 1)
        inv_dx = 1.0 / dx
        bias_g = -mins * inv_dx
        Y = np.concatenate([mins[:, None], ys[b].astype(np.float64), maxs[:, None]], 1)
        dY = np.diff(Y, 1)                                   # [8, 17]
        d = np.concatenate([dY[:, :1], np.diff(dY, axis=1)], 1)  # [8, 17]
        lin = dY[:, 0] + d[:, 1:9].sum(1)                    # two-sided fold
        koff = (d[:, 1:9] * np.arange(1, 9)).sum(1)
        const_c = pinv.T @ (Y[:, 0] - koff + lin * bias_g)   # [3]

        par = np.zeros((128, 11), np.float32)
        # partition p = h*64 + a*8 + j
        aidx = (np.arange(128) // J) % NA
        hidx = np.arange(128) // 64
        for r in range(8):
            # feature = relu(h - c): top c = 8.5-(r+1), bottom c = (r+9)-8.5
            cv = np.where(hidx == 0, 8.5 - (r + 1), (r + 9) - 8.5)
            if r in ACT_PAIRS:
                par[:, P_KV0 + r] = -cv  # ACT bias
            else:
                par[:, P_KV0 + r] = cv   # DVE subtract
        sgn = np.where(hidx == 0, -1.0, 1.0)
        par[:, P_HSC] = sgn * inv_dx[aidx]
        par[:, P_HBI] = sgn * (bias_g[aidx] - 8.5)
        cidx = (np.arange(128) % 32) // J
        par[:, P_BOUT] = const_c[np.minimum(cidx, 2)]

        A16 = Ab.astype(np.float16).astype(np.float64)
        wf = np.zeros((C * J, 128), np.float64)   # (c j) -> (h a j)
        for c in range(C):
            for j in range(J):
                for h in range(2):
                    for a in range(NA):
                        wf[c * J + j, h * 64 + a * J + j] = A16[c, a]
        wlin = np.einsum("ac,a,ka->kc", pinv, lin * inv_dx, Ab)  # [3, 3]
        wl = np.zeros((C * J, C * J), np.float64)
        for j in range(J):
            wl[j::J, j::J] = wlin
        wf4 = np.zeros((128, 128), np.float16)
        wlin4 = np.zeros((128, C * J), np.float16)
        for bk in range(NB):
            wf4[32 * bk : 32 * bk + C * J] = wf.astype(np.float16)
            wlin4[32 * bk : 32 * bk + C * J] = wl.astype(np.float16)

        wks = np.zeros((128, 8 * C * J), np.float16)
        for r in range(8):
            for h, k in ((0, r + 1), (1, r + 9)):
                for a in range(NA):
                    w_ac = d[a, k] * pinv[a]  # [3]
                    for j in range(J):
                        for c in range(C):
                            wks[h * 64 + a * J + j, r * C * J + c * J + j] = w_ac[c]

        rb = raw[b].reshape(C, J, NCOL).reshape(C * J, NCOL)
        in_maps.append(
            {
                "rawh": rb.astype(np.float16),
                "par": par,
                "wf4": wf4,
                "wlin4": wlin4,
                "wks": wks,
            }
        )
    return in_maps


def kernel(raw, ys, A):
    raw = np.asarray(raw, np.float32)
    ys = np.asarray(ys, np.float32)
    A = np.asarray(A, np.float32)
    if "nc" not in _NC_CACHE:
        _NC_CACHE["nc"] = _build_nc()
    nc = _NC_CACHE["nc"]
    in_maps = _host_params(raw, ys, A)
    res = run_bass_kernel_spmd(nc, in_maps, core_ids=list(range(B)))
    out = np.stack(
        [
            res.results[b]["out"]
            .astype(np.float32)
            .reshape(C, J, NCOL)
            .reshape(C, H, W)
            for b in range(B)
        ]
    )
    return out
